# revision 39
# baseline (speedup 1.0000x reference)
"""Trainium2 Bass kernel for the Involution module (B=8, C=256, H=W=56, K=7).

Strategy (8 NeuronCores, data-parallel over batch; one batch element per core):
  - conv1x1+BN+ReLU twice on the PE (bf16, BN folded into weights in numpy).
  - Involution: partitions = (group g:16, kj:7) = 112 lanes.
    x is pre-replicated 7x (kj-shifted copies) host-side and held fully
    resident in SBUF ([112, 16, 62, 56]); row-chunk DMAs are ordered so band
    0 starts as early as possible; zero pad rows are never loaded or
    multiplied (taps are row-clipped at the top/bottom image edge).
    Per band (224 pixels = 4 output rows): products pr[(g,kj), i, ki, hw] =
    wgt * x are computed on the DVE (bf16 2x mode, the bottleneck engine at
    ~100% steady-state duty), fused over ki groups (3,3,1) to amortize
    per-instruction overhead. PE reduces over kj with a 0/1 selection matmul
    (4-way col-tiled, 32-wide zero-padded stationary so all 128 psum
    partitions are written), accumulating the 7 ki taps in PSUM.
    The conv chain runs one band ahead of the products (so the ACT-queue
    order is relu(b+1)... copy(b) and the DVE never waits on the psum-copy
    chain). One ACT copy moves PSUM->SBUF bf16 per band and one DMA writes
    it out band-major (the last band splits copy+DMA per s-slot to shorten
    the tail); host decompacts and casts.
    Consts ride in two packed blobs: pkc1 carries everything the band-0
    conv chain needs (w1, w2 tap ki=2, selection matrix, biases) so the
    first products start ~14us in; pkc2 carries the remaining w2 taps.
"""

import numpy as np
import ml_dtypes

B, C, H, W = 8, 256, 56, 56
K = 7
GC = 16
G = 16
RED = 64
K2 = 49
EPS = 1e-5
HW = H * W            # 3136
PAD = 3
HP = H + 2 * PAD      # 62
WP = W + 2 * PAD      # 62
NB = 14               # bands
BW = HW // NB         # 224 columns per band (4 output rows)
BR = 4                # rows per band
NKJ = 7
NP = G * NKJ          # 112 partitions
KIF = ((0, 3), (3, 3), (6, 1))   # fused ki groups (start, len)
# SBUF/HBM row chunks of the resident x tile (padded-row indices); chunk c
# loads rows [XCH[c], XCH[c+1]). Pad rows (0:3, 59:62) are never touched.
XCH = (3, 6, 10, 14, 18, 26, 34, 42, 50, 59)
PK1W = 736            # packed part 1 (w1, w2-ki2, sel, biases, band-0 x_conv)
PK2W = 672            # packed-consts part 2 width (w2 for other kis)
KI2ORD = (2, 0, 1, 3, 4, 5, 6)   # w2 slot order in pkc1/pkc2

bf16 = ml_dtypes.bfloat16

_CACHE = {}


def _build_nc():
    import concourse.bacc as bacc
    import concourse.tile as tile
    from concourse import mybir

    f32 = mybir.dt.float32
    b16 = mybir.dt.bfloat16

    nc = bacc.Bacc("TRN2", target_bir_lowering=False, debug=False, num_devices=8)

    x_rep = nc.dram_tensor(
        "x_rep", [NP, GC, HP, W], b16, kind="ExternalInput"
    ).ap()
    x_conv = nc.dram_tensor("x_conv", [128, NB, 2, BW], b16, kind="ExternalInput").ap()
    pkc1 = nc.dram_tensor("pkc1", [128, PK1W], b16, kind="ExternalInput").ap()
    pkc2 = nc.dram_tensor("pkc2", [RED, PK2W], b16, kind="ExternalInput").ap()
    # out rows p = 32j+g (g<16; rows 32j+16..32j+31 are zero padding);
    # free (band, i4=2s+c, hw) with channel = 16g+4j+i4
    out = nc.dram_tensor("out", [128, NB, 4, BW], b16, kind="ExternalOutput").ap()

    with tile.TileContext(nc) as tc:
        _body(tc, nc, mybir, x_rep, x_conv, pkc1, pkc2, out)

    nc.compile()
    return nc


def _body(tc, nc, mybir, x_rep, x_conv, pkc1, pkc2, out):
    from concourse.bass import AP as BassAP

    f32 = mybir.dt.float32
    b16 = mybir.dt.bfloat16
    Relu = mybir.ActivationFunctionType.Relu
    mult = mybir.AluOpType.mult

    import contextlib
    ctx = contextlib.ExitStack()
    const = ctx.enter_context(tc.tile_pool(name="const", bufs=1))
    xcp = ctx.enter_context(tc.tile_pool(name="xcp", bufs=4))
    xrp = ctx.enter_context(tc.tile_pool(name="xrp", bufs=1))
    h1p = ctx.enter_context(tc.tile_pool(name="h1p", bufs=3))
    wgp = ctx.enter_context(tc.tile_pool(name="wgp", bufs=3))
    prp = ctx.enter_context(tc.tile_pool(name="prp", bufs=3))
    pcp = ctx.enter_context(tc.tile_pool(name="pcp", bufs=1))
    osp = ctx.enter_context(tc.tile_pool(name="osp", bufs=2))
    ph1 = ctx.enter_context(tc.tile_pool(name="ph1", bufs=2, space="PSUM"))
    ph2 = ctx.enter_context(tc.tile_pool(name="ph2", bufs=2, space="PSUM"))
    pho = ctx.enter_context(tc.tile_pool(name="pho", bufs=2, space="PSUM"))

    # resident x tile: [112, 16, 62, 56]; pad rows stay unwritten (never read)
    xrt = xrp.tile([NP, GC, HP, W], b16, tag="xr")

    def load_chunk(c):
        r0, r1 = XCH[c], XCH[c + 1]
        nc.sync.dma_start(out=xrt[:, :, r0:r1, :], in_=x_rep[:, :, r0:r1, :])

    def load_xcv(b):
        t = xcp.tile([128, 2, BW], b16, tag="xcv")
        nc.sync.dma_start(out=t[:], in_=x_conv[:, b, :, :])
        return t

    # All DMAs drain through one FIFO pipe in issue order, so issue order =
    # priority: packed consts + band-0 x_conv first (unblocks the conv
    # chain), then x row chunks just-in-time interleaved with later x_conv.
    pks1 = const.tile([128, PK1W], b16, tag="pks1")
    nc.sync.dma_start(out=pks1[:], in_=pkc1)
    w1s = pks1[:, 0:128].rearrange("p (a r) -> p a r", a=2)       # [128,2,64]
    sels = pks1[0:NP, 240:272]                                    # [112,32]
    b1s = pks1[0:RED, 272:274].bitcast(f32)                       # [64,1]
    b2s = pks1[0:NP, 274:288].bitcast(f32)                        # [112,7]

    # band-0 x_conv rides in the same first DMA as the conv weights
    xcv = {0: pks1[:, 288:736].rearrange("p (a n) -> p a n", a=2)}
    pks2 = const.tile([RED, PK2W], b16, tag="pks2")
    nc.sync.dma_start(out=pks2[:], in_=pkc2)

    def w2v(ki):
        # w2 stationary for tap ki: ki=2 rides in the first (small) const
        # DMA so the band-0 conv chain starts as early as possible
        i = KI2ORD.index(ki)
        if i == 0:
            return pks1[0:RED, 128:240]
        return pks2[0:RED, 112 * (i - 1):112 * i]
    load_chunk(0)  # rows 3:6   (band 0 clipped edge taps)
    xcv[1] = load_xcv(1)
    load_chunk(1)  # rows 6:10  (band 0 fused taps, band 1 first group)
    xcv[2] = load_xcv(2)
    load_chunk(2)  # rows 10:14 (band 1)
    xcv[3] = load_xcv(3)
    load_chunk(3)  # rows 14:18 (band 2)
    load_chunk(4)  # rows 18:26 (bands 3,4)

    def conv_band(b, ki_order=tuple(range(K))):
        # conv1: h1 = relu(W1' @ x + b1'); conv2 per ki:
        # wgt[(g,kj), ki, hw_band] = relu(W2'[ki] @ h1 + b2'[ki])
        xcb = xcv.pop(b)
        p1 = ph1.tile([RED, BW], f32, tag="p1")
        nc.tensor.matmul(p1[:], w1s[:, 0, :], xcb[:, 0, :], start=True, stop=False)
        nc.tensor.matmul(p1[:], w1s[:, 1, :], xcb[:, 1, :], start=False, stop=True)
        h1b = h1p.tile([RED, BW], b16, tag="h1b")
        nc.scalar.activation(h1b[:], p1[:], Relu, bias=b1s[:], scale=1.0)
        wgb = wgp.tile([NP, K, BW], b16, tag="wgb")
        for ki in ki_order:
            p2 = ph2.tile([NP, BW], f32, tag="p2")
            nc.tensor.matmul(p2[:], w2v(ki), h1b[:], start=True, stop=True)
            nc.scalar.activation(
                wgb[:, ki, :], p2[:], Relu, bias=b2s[:, ki:ki + 1], scale=1.0
            )
        return wgb

    def reduce_mm(po, src, pix0, pixn, start, stop, skip):
        # one kj-selection pass: po[(j,g), s, (c, pix)] += sel.T @ src
        # src: [NP, 2, pixn] i-pair product slice for out pixels
        # [pix0, pix0+pixn); po cols for chan c are at c*BW + pix.
        for p4 in range(8):
            j, s = p4 // 2, p4 % 2
            dst = po[32 * j:32 * j + 32, s, 0:2 * BW].rearrange(
                "p (c n) -> p c n", c=2)[:, :, pix0:pix0 + pixn]
            nc.tensor.matmul(
                dst,
                sels[:],
                src(p4),
                start=start,
                stop=stop,
                skip_group_check=skip,
                tile_position=(0, 32 * j),
            )

    def prod_fused(b, wg, po, k0, kl, start, stop, skip):
        r0 = BR * b + k0
        # x operand: overlapping window AP [(part),(i),(ki: row stride),
        # (pix: 4 contiguous rows)] — rows r0+kk .. r0+kk+3 per fused kk.
        v = xrt[:, :, r0:r0 + 1, :]
        xw = BassAP(
            v.tensor, v.offset,
            [list(v.ap[0]), list(v.ap[1]), [W, kl], [1, BW]],
        )
        pr = prp.tile([NP, GC, kl, BW], b16, tag="pr")
        nc.vector.tensor_tensor(
            out=pr[:],
            in0=wg[:, k0:k0 + kl, :].unsqueeze(1).broadcast_to(
                [NP, GC, kl, BW]),
            in1=xw,
            op=mult,
        )
        for kk in range(kl):
            src = lambda p4, kk=kk: pr[:, 2 * p4:2 * p4 + 2, kk, :]
            reduce_mm(po, src, 0, BW, start and kk == 0,
                      stop and kk == kl - 1, skip)

    def clip_products(b, wg, kis):
        # edge taps: multiply only the valid x rows. For band 0, tap ki<3
        # covers out pixels [(3-ki)*56, 224) from x rows 3..3+ki; for band
        # 13, tap ki>3 covers out pixels [0, (7-ki)*56) from x rows
        # 52+ki..58. Returns deferred reduce items (their matmuls must
        # execute after the band's full-range start=True matmul: psum
        # start resets the whole bank).
        pr = pcp.tile([NP, GC, 6 * W], b16, tag="prc")
        items = []
        off = 0
        for ki in kis:
            if b == 0:
                rows, r0, pix0 = ki + 1, PAD, (PAD - ki) * W
            else:
                rows, r0, pix0 = K - ki, BR * b + ki, 0
            pixn = rows * W
            v = xrt[:, :, r0:r0 + 1, :]
            xw = BassAP(
                v.tensor, v.offset,
                [list(v.ap[0]), list(v.ap[1]), [1, pixn]],
            )
            nc.vector.tensor_tensor(
                out=pr[:, :, off:off + pixn],
                in0=wg[:, ki, pix0:pix0 + pixn].unsqueeze(1).broadcast_to(
                    [NP, GC, pixn]),
                in1=xw,
                op=mult,
            )
            items.append((off, pixn, pix0, ki))
            off += pixn
        return pr, items

    def clip_reduces(po, pr, items, stop_ki):
        for off, pixn, pix0, ki in items:
            reduce_mm(
                po, lambda p4, o=off, n=pixn: pr[:, 2 * p4:2 * p4 + 2, o:o + n],
                pix0, pixn, False, ki == stop_ki, True,
            )

    wg_cur = conv_band(0, ki_order=(2, 1, 0, 3, 4, 5, 6))

    for b in range(NB):
        # stream upcoming inputs (issue order = FIFO priority)
        if b + 4 < NB:
            xcv[b + 4] = load_xcv(b + 4)
        if 5 + b < len(XCH) - 1:
            load_chunk(5 + b)

        # conv chain one band ahead: its ACT relus precede copy(b) in the
        # ACT queue, so band b+1's products never wait on the psum copy.
        wg_next = conv_band(b + 1) if b + 1 < NB else None

        # involution: ki-fused DVE products + kj reduction via selection
        # matmul; ki accumulated in PSUM.
        # i-pair p4 = i//2: strip j = p4//2 (psum partitions 32j..32j+32),
        # slot s = p4%2. channel i = 4j + 2s + c.
        po = pho.tile([128, 2, 512], f32, tag="po")  # s-slot padded to a PSUM bank

        if b == 0:
            # clipped-tap products first on the DVE (they need only x rows
            # 3:6, the first chunk); their psum reduces are deferred until
            # after the fused group's full-range start=True.
            prc, items = clip_products(b, wg_cur, (2, 1, 0))
            prod_fused(b, wg_cur, po, 3, 3, True, False, True)
            clip_reduces(po, prc, items, None)
            prod_fused(b, wg_cur, po, 6, 1, False, True, True)
        elif b == NB - 1:
            prod_fused(b, wg_cur, po, 0, 3, True, False, True)
            prod_fused(b, wg_cur, po, 3, 1, False, False, True)
            prc, items = clip_products(b, wg_cur, (4, 5, 6))
        else:
            for k0, kl in KIF:
                prod_fused(b, wg_cur, po, k0, kl, k0 == 0, k0 + kl == K, False)

        # PSUM -> SBUF bf16 in one 128-partition copy (psum partitions
        # 16..31 etc hold zeros from the padded stationary), then DMA
        # compacts the 4 quadrants to HBM.
        ob = osp.tile([128, 2, 2 * BW], b16, tag="ob")
        if b < NB - 1:
            nc.scalar.copy(out=ob[:], in_=po[:, :, 0:2 * BW])
            # single out DMA of all 128 partitions (16-row pad groups
            # included; host discards them) — one issue instead of four.
            nc.sync.dma_start(
                out=out[:, b, :, :],
                in_=ob[:].rearrange("p s (c n) -> p (s c) n", c=2),
            )
        else:
            # last band: psum pixel columns finalize progressively (pixels
            # [168,224) after full tap ki=3, then one more 56-px row per
            # clipped tap), so pipeline copy+DMA per row — only the last
            # 56-px slice remains after the final product.
            def out_slice(a, n):
                obv = ob[:].rearrange("p s (c n) -> p s c n", c=2)
                pov = po[:, :, 0:2 * BW].rearrange("p s (c n) -> p s c n", c=2)
                nc.scalar.copy(out=obv[:, :, :, a:a + n],
                               in_=pov[:, :, :, a:a + n])
                nc.sync.dma_start(
                    out=out[:, b, :, a:a + n],
                    in_=obv[:, :, :, a:a + n].rearrange("p s c n -> p (s c) n"),
                )
            out_slice(3 * W, W)
            for off, pixn, pix0, ki in items:
                reduce_mm(
                    po,
                    lambda p4, o=off, m=pixn: prc[:, 2 * p4:2 * p4 + 2, o:o + m],
                    pix0, pixn, False, ki == 6, True,
                )
                out_slice((6 - ki) * W, W)

        wg_cur = wg_next

    ctx.close()


def _prep_weights(w1, b1, g1, be1, m1, v1, w2, b2, g2, be2, m2, v2):
    s1 = (g1 / np.sqrt(v1 + EPS)).astype(np.float64)
    W1p = w1.astype(np.float64) * s1[:, None]
    b1p = be1 + (b1 - m1) * (g1 / np.sqrt(v1 + EPS))
    s2 = (g2 / np.sqrt(v2 + EPS)).astype(np.float64)
    W2p = w2.astype(np.float64) * s2[:, None]
    b2p = be2 + (b2 - m2) * (g2 / np.sqrt(v2 + EPS))

    w1t = np.ascontiguousarray(
        W1p.astype(np.float32).T.reshape(2, 128, RED).transpose(1, 0, 2)
    ).astype(bf16)
    # w2t[r, ki, 7g+kj] = W2p[g*49 + ki*7 + kj, r]
    w2t = np.ascontiguousarray(
        W2p.astype(np.float32).reshape(G, K, K, RED).transpose(3, 1, 0, 2).reshape(RED, K, NP)
    ).astype(bf16)
    b2t = np.ascontiguousarray(
        b2p.astype(np.float32).reshape(G, K, K).transpose(0, 2, 1).reshape(NP, K)
    )
    selm = np.zeros((NP, 32), np.float32)
    selm[:, :G] = np.repeat(np.eye(G, dtype=np.float32), NKJ, axis=0)

    # pack consts into two bf16 blobs (f32 payloads bitcast): pkc1 has
    # everything the band-0 conv chain needs, pkc2 the remaining w2 taps
    pk1 = np.zeros((128, 288), bf16)
    pk1[:, 0:128] = w1t.reshape(128, 128)
    pk1[0:RED, 128:240] = w2t[:, KI2ORD[0], :]
    pk1[0:NP, 240:272] = selm.astype(bf16)
    pk1[0:RED, 272:274] = b1p.astype(np.float32).reshape(RED, 1).view(bf16)
    pk1[0:NP, 274:288] = b2t.view(bf16)
    pk2 = np.zeros((RED, PK2W), bf16)
    for i, ki in enumerate(KI2ORD[1:]):
        pk2[:, 112 * i:112 * (i + 1)] = w2t[:, ki, :]
    return pk1, pk2


def _prep_core(xc):
    """xc: [C, H, W] fp32 -> (x_rep bf16 [112,16,62,56] kj-shifted copies,
    x_conv bf16 [128,NB,2,BW])"""
    xw = np.zeros((G, GC, HP, WP), np.float32)
    xw[:, :, PAD:PAD + H, PAD:PAD + W] = xc.reshape(G, GC, H, W)
    arr = np.empty((G, NKJ, GC, HP, W), np.float32)
    for kj in range(NKJ):
        arr[:, kj] = xw[:, :, :, kj:kj + W]
    x_rep = arr.reshape(NP, GC, HP, W).astype(bf16)
    x_conv = np.ascontiguousarray(
        xc.reshape(2, 128, NB, BW).transpose(1, 2, 0, 3)
    ).astype(bf16)
    return x_rep, x_conv


def kernel(x, w1, b1, g1, be1, m1, v1, w2, b2, g2, be2, m2, v2, _profile=False):
    from concourse.bass_utils import run_bass_kernel_spmd

    if "nc" not in _CACHE:
        _CACHE["nc"] = _build_nc()
    nc = _CACHE["nc"]

    x = np.asarray(x, np.float32)
    pk1, pk2 = _prep_weights(
        np.asarray(w1, np.float32), np.asarray(b1, np.float32),
        np.asarray(g1, np.float32), np.asarray(be1, np.float32),
        np.asarray(m1, np.float32), np.asarray(v1, np.float32),
        np.asarray(w2, np.float32), np.asarray(b2, np.float32),
        np.asarray(g2, np.float32), np.asarray(be2, np.float32),
        np.asarray(m2, np.float32), np.asarray(v2, np.float32),
    )

    in_maps = []
    for c in range(B):
        x_rep, x_conv = _prep_core(x[c].reshape(C, H, W))
        pk1c = np.concatenate(
            [pk1, np.ascontiguousarray(x_conv[:, 0]).reshape(128, 448)],
            axis=1)
        in_maps.append({"x_rep": x_rep, "x_conv": x_conv,
                        "pkc1": pk1c, "pkc2": pk2})

    res = run_bass_kernel_spmd(
        nc, in_maps, core_ids=list(range(8)), trace=_profile
    )
    outs = []
    for c in range(B):
        arr = res.results[c]["out"].astype(np.float32)  # [128, NB, 4, BW], row p = 32j+g
        arr = arr.reshape(4, 32, NB, 4, BW)[:, :G]     # [j, g, band, i4, n]
        arr = arr.transpose(1, 0, 3, 2, 4).reshape(C, HW)  # channel = 16g+4j+i4
        outs.append(arr)
    outp = np.stack(outs, axis=0)
    if _profile:
        _CACHE["last_result"] = res
    return outp.reshape(B, C, H, W).astype(np.float32)


# revision 40
# speedup vs baseline: 1.0050x; 1.0050x over previous
"""Trainium2 Bass kernel for the Involution module (B=8, C=256, H=W=56, K=7).

Strategy (8 NeuronCores, data-parallel over batch; one batch element per core):
  - conv1x1+BN+ReLU twice on the PE (bf16, BN folded into weights in numpy).
  - Involution: partitions = (group g:16, kj:7) = 112 lanes.
    x is pre-replicated 7x (kj-shifted copies) host-side and held fully
    resident in SBUF ([112, 16, 62, 56]); row-chunk DMAs are ordered so band
    0 starts as early as possible; zero pad rows are never loaded or
    multiplied (taps are row-clipped at the top/bottom image edge).
    Per band (224 pixels = 4 output rows): products pr[(g,kj), i, ki, hw] =
    wgt * x are computed on the DVE (bf16 2x mode, the bottleneck engine at
    ~100% steady-state duty), fused over ki groups (3,3,1) to amortize
    per-instruction overhead. PE reduces over kj with a 0/1 selection matmul
    (4-way col-tiled, 32-wide zero-padded stationary so all 128 psum
    partitions are written), accumulating the 7 ki taps in PSUM.
    The conv chain runs one band ahead of the products (so the ACT-queue
    order is relu(b+1)... copy(b) and the DVE never waits on the psum-copy
    chain). One ACT copy moves PSUM->SBUF bf16 per band and one DMA writes
    it out band-major (the last band splits copy+DMA per s-slot to shorten
    the tail); host decompacts and casts.
    Consts ride in two packed blobs: pkc1 carries everything the band-0
    conv chain needs (w1, w2 tap ki=2, selection matrix, biases) so the
    first products start ~14us in; pkc2 carries the remaining w2 taps.
"""

import numpy as np
import ml_dtypes

B, C, H, W = 8, 256, 56, 56
K = 7
GC = 16
G = 16
RED = 64
K2 = 49
EPS = 1e-5
HW = H * W            # 3136
PAD = 3
HP = H + 2 * PAD      # 62
WP = W + 2 * PAD      # 62
NB = 14               # bands
BW = HW // NB         # 224 columns per band (4 output rows)
BR = 4                # rows per band
NKJ = 7
NP = G * NKJ          # 112 partitions
KIF = ((0, 3), (3, 3), (6, 1))   # fused ki groups (start, len)
# SBUF/HBM row chunks of the resident x tile (padded-row indices); chunk c
# loads rows [XCH[c], XCH[c+1]). Pad rows (0:3, 59:62) are never touched.
XCH = (3, 6, 10, 14, 18, 26, 34, 42, 50, 59)
PK1W = 736            # packed part 1 (w1, w2-ki2, sel, biases, band-0 x_conv)
PK2W = 672            # packed-consts part 2 width (w2 for other kis)
KI2ORD = (2, 0, 1, 3, 4, 5, 6)   # w2 slot order in pkc1/pkc2

bf16 = ml_dtypes.bfloat16

_CACHE = {}


def _build_nc():
    import concourse.bacc as bacc
    import concourse.tile as tile
    from concourse import mybir

    f32 = mybir.dt.float32
    b16 = mybir.dt.bfloat16

    nc = bacc.Bacc("TRN2", target_bir_lowering=False, debug=False, num_devices=8)

    x_rep = nc.dram_tensor(
        "x_rep", [NP, GC, HP, W], b16, kind="ExternalInput"
    ).ap()
    x_conv = nc.dram_tensor("x_conv", [128, NB, 2, BW], b16, kind="ExternalInput").ap()
    pkc1 = nc.dram_tensor("pkc1", [128, PK1W], b16, kind="ExternalInput").ap()
    pkc2 = nc.dram_tensor("pkc2", [RED, PK2W], b16, kind="ExternalInput").ap()
    # out rows p = 32j+g (g<16; rows 32j+16..32j+31 are zero padding);
    # free (band, i4=2s+c, hw) with channel = 16g+4j+i4
    out = nc.dram_tensor("out", [128, NB, 4, BW], b16, kind="ExternalOutput").ap()

    with tile.TileContext(nc) as tc:
        _body(tc, nc, mybir, x_rep, x_conv, pkc1, pkc2, out)

    nc.compile()
    return nc


def _body(tc, nc, mybir, x_rep, x_conv, pkc1, pkc2, out):
    from concourse.bass import AP as BassAP

    f32 = mybir.dt.float32
    b16 = mybir.dt.bfloat16
    Relu = mybir.ActivationFunctionType.Relu
    mult = mybir.AluOpType.mult

    import contextlib
    ctx = contextlib.ExitStack()
    const = ctx.enter_context(tc.tile_pool(name="const", bufs=1))
    xcp = ctx.enter_context(tc.tile_pool(name="xcp", bufs=4))
    xrp = ctx.enter_context(tc.tile_pool(name="xrp", bufs=1))
    h1p = ctx.enter_context(tc.tile_pool(name="h1p", bufs=3))
    wgp = ctx.enter_context(tc.tile_pool(name="wgp", bufs=3))
    prp = ctx.enter_context(tc.tile_pool(name="prp", bufs=3))
    pcp = ctx.enter_context(tc.tile_pool(name="pcp", bufs=1))
    osp = ctx.enter_context(tc.tile_pool(name="osp", bufs=2))
    ph1 = ctx.enter_context(tc.tile_pool(name="ph1", bufs=2, space="PSUM"))
    ph2 = ctx.enter_context(tc.tile_pool(name="ph2", bufs=2, space="PSUM"))
    pho = ctx.enter_context(tc.tile_pool(name="pho", bufs=2, space="PSUM"))

    # resident x tile: [112, 16, 62, 56]; pad rows stay unwritten (never read)
    xrt = xrp.tile([NP, GC, HP, W], b16, tag="xr")

    def load_chunk(c):
        r0, r1 = XCH[c], XCH[c + 1]
        nc.sync.dma_start(out=xrt[:, :, r0:r1, :], in_=x_rep[:, :, r0:r1, :])

    def load_xcv(b):
        t = xcp.tile([128, 2, BW], b16, tag="xcv")
        nc.sync.dma_start(out=t[:], in_=x_conv[:, b, :, :])
        return t

    # All DMAs drain through one FIFO pipe in issue order, so issue order =
    # priority: packed consts + band-0 x_conv first (unblocks the conv
    # chain), then x row chunks just-in-time interleaved with later x_conv.
    pks1 = const.tile([128, PK1W], b16, tag="pks1")
    nc.sync.dma_start(out=pks1[:], in_=pkc1)
    w1s = pks1[:, 0:128].rearrange("p (a r) -> p a r", a=2)       # [128,2,64]
    sels = pks1[0:NP, 240:272]                                    # [112,32]
    b1s = pks1[0:RED, 272:274].bitcast(f32)                       # [64,1]
    b2s = pks1[0:NP, 274:288].bitcast(f32)                        # [112,7]

    # band-0 x_conv rides in the same first DMA as the conv weights
    xcv = {0: pks1[:, 288:736].rearrange("p (a n) -> p a n", a=2)}
    pks2 = const.tile([RED, PK2W], b16, tag="pks2")
    nc.sync.dma_start(out=pks2[:], in_=pkc2)

    def w2v(ki):
        # w2 stationary for tap ki: ki=2 rides in the first (small) const
        # DMA so the band-0 conv chain starts as early as possible
        i = KI2ORD.index(ki)
        if i == 0:
            return pks1[0:RED, 128:240]
        return pks2[0:RED, 112 * (i - 1):112 * i]
    load_chunk(0)  # rows 3:6   (band 0 clipped edge taps)
    xcv[1] = load_xcv(1)
    load_chunk(1)  # rows 6:10  (band 0 fused taps, band 1 first group)
    xcv[2] = load_xcv(2)
    load_chunk(2)  # rows 10:14 (band 1)
    xcv[3] = load_xcv(3)
    load_chunk(3)  # rows 14:18 (band 2)
    load_chunk(4)  # rows 18:26 (bands 3,4)

    def conv_band(b, ki_order=tuple(range(K))):
        # conv1: h1 = relu(W1' @ x + b1'); conv2 per ki:
        # wgt[(g,kj), ki, hw_band] = relu(W2'[ki] @ h1 + b2'[ki])
        xcb = xcv.pop(b)
        p1 = ph1.tile([RED, BW], f32, tag="p1")
        nc.tensor.matmul(p1[:], w1s[:, 0, :], xcb[:, 0, :], start=True, stop=False)
        nc.tensor.matmul(p1[:], w1s[:, 1, :], xcb[:, 1, :], start=False, stop=True)
        h1b = h1p.tile([RED, BW], b16, tag="h1b")
        nc.scalar.activation(h1b[:], p1[:], Relu, bias=b1s[:], scale=1.0)
        wgb = wgp.tile([NP, K, BW], b16, tag="wgb")
        for ki in ki_order:
            p2 = ph2.tile([NP, BW], f32, tag="p2")
            nc.tensor.matmul(p2[:], w2v(ki), h1b[:], start=True, stop=True)
            nc.scalar.activation(
                wgb[:, ki, :], p2[:], Relu, bias=b2s[:, ki:ki + 1], scale=1.0
            )
        return wgb

    def reduce_mm(po, src, pix0, pixn, start, stop, skip):
        # one kj-selection pass: po[(j,g), s, (c, pix)] += sel.T @ src
        # src: [NP, 2, pixn] i-pair product slice for out pixels
        # [pix0, pix0+pixn); po cols for chan c are at c*BW + pix.
        for p4 in range(8):
            j, s = p4 // 2, p4 % 2
            dst = po[32 * j:32 * j + 32, s, 0:2 * BW].rearrange(
                "p (c n) -> p c n", c=2)[:, :, pix0:pix0 + pixn]
            nc.tensor.matmul(
                dst,
                sels[:],
                src(p4),
                start=start,
                stop=stop,
                skip_group_check=skip,
                tile_position=(0, 32 * j),
            )

    def prod_fused(b, wg, po, k0, kl, start, stop, skip):
        r0 = BR * b + k0
        # x operand: overlapping window AP [(part),(i),(ki: row stride),
        # (pix: 4 contiguous rows)] — rows r0+kk .. r0+kk+3 per fused kk.
        v = xrt[:, :, r0:r0 + 1, :]
        xw = BassAP(
            v.tensor, v.offset,
            [list(v.ap[0]), list(v.ap[1]), [W, kl], [1, BW]],
        )
        pr = prp.tile([NP, GC, kl, BW], b16, tag="pr")
        nc.vector.tensor_tensor(
            out=pr[:],
            in0=wg[:, k0:k0 + kl, :].unsqueeze(1).broadcast_to(
                [NP, GC, kl, BW]),
            in1=xw,
            op=mult,
        )
        for kk in range(kl):
            src = lambda p4, kk=kk: pr[:, 2 * p4:2 * p4 + 2, kk, :]
            reduce_mm(po, src, 0, BW, start and kk == 0,
                      stop and kk == kl - 1, skip)

    def clip_products(b, wg, kis):
        # edge taps: multiply only the valid x rows. For band 0, tap ki<3
        # covers out pixels [(3-ki)*56, 224) from x rows 3..3+ki; for band
        # 13, tap ki>3 covers out pixels [0, (7-ki)*56) from x rows
        # 52+ki..58. Returns deferred reduce items (their matmuls must
        # execute after the band's full-range start=True matmul: psum
        # start resets the whole bank).
        pr = pcp.tile([NP, GC, 6 * W], b16, tag="prc")
        items = []
        off = 0
        for ki in kis:
            if b == 0:
                rows, r0, pix0 = ki + 1, PAD, (PAD - ki) * W
            else:
                rows, r0, pix0 = K - ki, BR * b + ki, 0
            pixn = rows * W
            v = xrt[:, :, r0:r0 + 1, :]
            xw = BassAP(
                v.tensor, v.offset,
                [list(v.ap[0]), list(v.ap[1]), [1, pixn]],
            )
            nc.vector.tensor_tensor(
                out=pr[:, :, off:off + pixn],
                in0=wg[:, ki, pix0:pix0 + pixn].unsqueeze(1).broadcast_to(
                    [NP, GC, pixn]),
                in1=xw,
                op=mult,
            )
            items.append((off, pixn, pix0, ki))
            off += pixn
        return pr, items

    def clip_reduces(po, pr, items, stop_ki):
        for off, pixn, pix0, ki in items:
            reduce_mm(
                po, lambda p4, o=off, n=pixn: pr[:, 2 * p4:2 * p4 + 2, o:o + n],
                pix0, pixn, False, ki == stop_ki, True,
            )

    wg_cur = conv_band(0, ki_order=(2, 1, 0, 3, 4, 5, 6))

    for b in range(NB):
        # stream upcoming inputs (issue order = FIFO priority)
        if b + 4 < NB:
            xcv[b + 4] = load_xcv(b + 4)
        if 5 + b < len(XCH) - 1:
            load_chunk(5 + b)

        # conv chain one band ahead: its ACT relus precede copy(b) in the
        # ACT queue, so band b+1's products never wait on the psum copy.
        wg_next = conv_band(b + 1) if b + 1 < NB else None

        # involution: ki-fused DVE products + kj reduction via selection
        # matmul; ki accumulated in PSUM.
        # i-pair p4 = i//2: strip j = p4//2 (psum partitions 32j..32j+32),
        # slot s = p4%2. channel i = 4j + 2s + c.
        po = pho.tile([128, 2, 512], f32, tag="po")  # s-slot padded to a PSUM bank

        if b == 0:
            # clipped-tap products first on the DVE (they need only x rows
            # 3:6, the first chunk); their psum reduces are deferred until
            # after the fused group's full-range start=True.
            prc, items = clip_products(b, wg_cur, (2, 1, 0))
            prod_fused(b, wg_cur, po, 3, 3, True, False, True)
            clip_reduces(po, prc, items, None)
            prod_fused(b, wg_cur, po, 6, 1, False, True, True)
        elif b == NB - 1:
            prod_fused(b, wg_cur, po, 0, 3, True, False, True)
            prod_fused(b, wg_cur, po, 3, 1, False, False, True)
            prc, items = clip_products(b, wg_cur, (4, 5, 6))
            clip_reduces(po, prc, items, 6)
        else:
            for k0, kl in KIF:
                prod_fused(b, wg_cur, po, k0, kl, k0 == 0, k0 + kl == K, False)

        # PSUM -> SBUF bf16 in one 128-partition copy (psum partitions
        # 16..31 etc hold zeros from the padded stationary), then DMA
        # compacts the 4 quadrants to HBM.
        ob = osp.tile([128, 2, 2 * BW], b16, tag="ob")
        if b < NB - 1:
            nc.scalar.copy(out=ob[:], in_=po[:, :, 0:2 * BW])
            # single out DMA of all 128 partitions (16-row pad groups
            # included; host discards them) — one issue instead of four.
            nc.sync.dma_start(
                out=out[:, b, :, :],
                in_=ob[:].rearrange("p s (c n) -> p (s c) n", c=2),
            )
        else:
            # last band: split copy+DMA per s-slot so the first half's
            # transfer overlaps the second half's copy (shorter tail)
            for s in range(2):
                nc.scalar.copy(out=ob[:, s, :], in_=po[:, s, 0:2 * BW])
                nc.sync.dma_start(
                    out=out[:, b, 2 * s:2 * s + 2, :],
                    in_=ob[:, s, :].rearrange("p (c n) -> p c n", c=2),
                )

        wg_cur = wg_next

    ctx.close()


def _prep_weights(w1, b1, g1, be1, m1, v1, w2, b2, g2, be2, m2, v2):
    s1 = (g1 / np.sqrt(v1 + EPS)).astype(np.float64)
    W1p = w1.astype(np.float64) * s1[:, None]
    b1p = be1 + (b1 - m1) * (g1 / np.sqrt(v1 + EPS))
    s2 = (g2 / np.sqrt(v2 + EPS)).astype(np.float64)
    W2p = w2.astype(np.float64) * s2[:, None]
    b2p = be2 + (b2 - m2) * (g2 / np.sqrt(v2 + EPS))

    w1t = np.ascontiguousarray(
        W1p.astype(np.float32).T.reshape(2, 128, RED).transpose(1, 0, 2)
    ).astype(bf16)
    # w2t[r, ki, 7g+kj] = W2p[g*49 + ki*7 + kj, r]
    w2t = np.ascontiguousarray(
        W2p.astype(np.float32).reshape(G, K, K, RED).transpose(3, 1, 0, 2).reshape(RED, K, NP)
    ).astype(bf16)
    b2t = np.ascontiguousarray(
        b2p.astype(np.float32).reshape(G, K, K).transpose(0, 2, 1).reshape(NP, K)
    )
    selm = np.zeros((NP, 32), np.float32)
    selm[:, :G] = np.repeat(np.eye(G, dtype=np.float32), NKJ, axis=0)

    # pack consts into two bf16 blobs (f32 payloads bitcast): pkc1 has
    # everything the band-0 conv chain needs, pkc2 the remaining w2 taps
    pk1 = np.zeros((128, 288), bf16)
    pk1[:, 0:128] = w1t.reshape(128, 128)
    pk1[0:RED, 128:240] = w2t[:, KI2ORD[0], :]
    pk1[0:NP, 240:272] = selm.astype(bf16)
    pk1[0:RED, 272:274] = b1p.astype(np.float32).reshape(RED, 1).view(bf16)
    pk1[0:NP, 274:288] = b2t.view(bf16)
    pk2 = np.zeros((RED, PK2W), bf16)
    for i, ki in enumerate(KI2ORD[1:]):
        pk2[:, 112 * i:112 * (i + 1)] = w2t[:, ki, :]
    return pk1, pk2


def _prep_core(xc):
    """xc: [C, H, W] fp32 -> (x_rep bf16 [112,16,62,56] kj-shifted copies,
    x_conv bf16 [128,NB,2,BW])"""
    xw = np.zeros((G, GC, HP, WP), np.float32)
    xw[:, :, PAD:PAD + H, PAD:PAD + W] = xc.reshape(G, GC, H, W)
    arr = np.empty((G, NKJ, GC, HP, W), np.float32)
    for kj in range(NKJ):
        arr[:, kj] = xw[:, :, :, kj:kj + W]
    x_rep = arr.reshape(NP, GC, HP, W).astype(bf16)
    x_conv = np.ascontiguousarray(
        xc.reshape(2, 128, NB, BW).transpose(1, 2, 0, 3)
    ).astype(bf16)
    return x_rep, x_conv


def kernel(x, w1, b1, g1, be1, m1, v1, w2, b2, g2, be2, m2, v2, _profile=False):
    from concourse.bass_utils import run_bass_kernel_spmd

    if "nc" not in _CACHE:
        _CACHE["nc"] = _build_nc()
    nc = _CACHE["nc"]

    x = np.asarray(x, np.float32)
    pk1, pk2 = _prep_weights(
        np.asarray(w1, np.float32), np.asarray(b1, np.float32),
        np.asarray(g1, np.float32), np.asarray(be1, np.float32),
        np.asarray(m1, np.float32), np.asarray(v1, np.float32),
        np.asarray(w2, np.float32), np.asarray(b2, np.float32),
        np.asarray(g2, np.float32), np.asarray(be2, np.float32),
        np.asarray(m2, np.float32), np.asarray(v2, np.float32),
    )

    in_maps = []
    for c in range(B):
        x_rep, x_conv = _prep_core(x[c].reshape(C, H, W))
        pk1c = np.concatenate(
            [pk1, np.ascontiguousarray(x_conv[:, 0]).reshape(128, 448)],
            axis=1)
        in_maps.append({"x_rep": x_rep, "x_conv": x_conv,
                        "pkc1": pk1c, "pkc2": pk2})

    res = run_bass_kernel_spmd(
        nc, in_maps, core_ids=list(range(8)), trace=_profile
    )
    outs = []
    for c in range(B):
        arr = res.results[c]["out"].astype(np.float32)  # [128, NB, 4, BW], row p = 32j+g
        arr = arr.reshape(4, 32, NB, 4, BW)[:, :G]     # [j, g, band, i4, n]
        arr = arr.transpose(1, 0, 3, 2, 4).reshape(C, HW)  # channel = 16g+4j+i4
        outs.append(arr)
    outp = np.stack(outs, axis=0)
    if _profile:
        _CACHE["last_result"] = res
    return outp.reshape(B, C, H, W).astype(np.float32)


# revision 41
# speedup vs baseline: 1.0051x; 1.0002x over previous
"""Trainium2 Bass kernel for the Involution module (B=8, C=256, H=W=56, K=7).

Strategy (8 NeuronCores, data-parallel over batch; one batch element per core):
  - conv1x1+BN+ReLU twice on the PE (bf16, BN folded into weights in numpy).
  - Involution: partitions = (group g:16, kj:7) = 112 lanes.
    x is pre-replicated 7x (kj-shifted copies) host-side and held fully
    resident in SBUF ([112, 16, 62, 56]); row-chunk DMAs are ordered so band
    0 starts as early as possible; zero pad rows are never loaded or
    multiplied (taps are row-clipped at the top/bottom image edge).
    Per band (224 pixels = 4 output rows): products pr[(g,kj), i, ki, hw] =
    wgt * x are computed on the DVE (bf16 2x mode, the bottleneck engine at
    ~100% steady-state duty), fused over ki groups (3,3,1) to amortize
    per-instruction overhead. PE reduces over kj with a 0/1 selection matmul
    (4-way col-tiled, 32-wide zero-padded stationary so all 128 psum
    partitions are written), accumulating the 7 ki taps in PSUM.
    The conv chain runs one band ahead of the products (so the ACT-queue
    order is relu(b+1)... copy(b) and the DVE never waits on the psum-copy
    chain). One ACT copy moves PSUM->SBUF bf16 per band and one DMA writes
    it out band-major (the last band splits copy+DMA per s-slot to shorten
    the tail); host decompacts and casts.
    Consts ride in two packed blobs: pkc1 carries everything the band-0
    conv chain needs (w1, w2 tap ki=2, selection matrix, biases) so the
    first products start ~14us in; pkc2 carries the remaining w2 taps.
"""

import numpy as np
import ml_dtypes

B, C, H, W = 8, 256, 56, 56
K = 7
GC = 16
G = 16
RED = 64
K2 = 49
EPS = 1e-5
HW = H * W            # 3136
PAD = 3
HP = H + 2 * PAD      # 62
WP = W + 2 * PAD      # 62
NB = 14               # bands
BW = HW // NB         # 224 columns per band (4 output rows)
BR = 4                # rows per band
NKJ = 7
NP = G * NKJ          # 112 partitions
KIF = ((0, 3), (3, 3), (6, 1))   # fused ki groups (start, len)
# SBUF/HBM row chunks of the resident x tile (padded-row indices); chunk c
# loads rows [XCH[c], XCH[c+1]). Pad rows (0:3, 59:62) are never touched.
XCH = (3, 6, 10, 14, 18, 26, 34, 42, 50, 59)
PK1W = 736            # packed part 1 (w1, w2-ki2, sel, biases, band-0 x_conv)
PK2W = 672            # packed-consts part 2 width (w2 for other kis)
KI2ORD = (2, 0, 1, 3, 4, 5, 6)   # w2 slot order in pkc1/pkc2

bf16 = ml_dtypes.bfloat16

_CACHE = {}


def _build_nc():
    import concourse.bacc as bacc
    import concourse.tile as tile
    from concourse import mybir

    f32 = mybir.dt.float32
    b16 = mybir.dt.bfloat16

    nc = bacc.Bacc("TRN2", target_bir_lowering=False, debug=False, num_devices=8)

    x_rep = nc.dram_tensor(
        "x_rep", [NP, GC, HP, W], b16, kind="ExternalInput"
    ).ap()
    x_conv = nc.dram_tensor("x_conv", [128, NB, 2, BW], b16, kind="ExternalInput").ap()
    pkc1 = nc.dram_tensor("pkc1", [128, PK1W], b16, kind="ExternalInput").ap()
    pkc2 = nc.dram_tensor("pkc2", [RED, PK2W], b16, kind="ExternalInput").ap()
    # out rows p = 32j+g (g<16; rows 32j+16..32j+31 are zero padding);
    # free (band, i4=2s+c, hw) with channel = 16g+4j+i4
    out = nc.dram_tensor("out", [128, NB, 4, BW], b16, kind="ExternalOutput").ap()

    with tile.TileContext(nc) as tc:
        _body(tc, nc, mybir, x_rep, x_conv, pkc1, pkc2, out)

    nc.compile()
    return nc


def _body(tc, nc, mybir, x_rep, x_conv, pkc1, pkc2, out):
    from concourse.bass import AP as BassAP

    f32 = mybir.dt.float32
    b16 = mybir.dt.bfloat16
    Relu = mybir.ActivationFunctionType.Relu
    mult = mybir.AluOpType.mult

    import contextlib
    ctx = contextlib.ExitStack()
    const = ctx.enter_context(tc.tile_pool(name="const", bufs=1))
    xcp = ctx.enter_context(tc.tile_pool(name="xcp", bufs=4))
    xrp = ctx.enter_context(tc.tile_pool(name="xrp", bufs=1))
    h1p = ctx.enter_context(tc.tile_pool(name="h1p", bufs=3))
    wgp = ctx.enter_context(tc.tile_pool(name="wgp", bufs=3))
    prp = ctx.enter_context(tc.tile_pool(name="prp", bufs=3))
    pcp = ctx.enter_context(tc.tile_pool(name="pcp", bufs=1))
    osp = ctx.enter_context(tc.tile_pool(name="osp", bufs=2))
    ph1 = ctx.enter_context(tc.tile_pool(name="ph1", bufs=2, space="PSUM"))
    ph2 = ctx.enter_context(tc.tile_pool(name="ph2", bufs=2, space="PSUM"))
    pho = ctx.enter_context(tc.tile_pool(name="pho", bufs=2, space="PSUM"))

    # resident x tile: [112, 16, 62, 56]; pad rows stay unwritten (never read)
    xrt = xrp.tile([NP, GC, HP, W], b16, tag="xr")

    def load_chunk(c):
        r0, r1 = XCH[c], XCH[c + 1]
        nc.sync.dma_start(out=xrt[:, :, r0:r1, :], in_=x_rep[:, :, r0:r1, :])

    def load_xcv(b):
        t = xcp.tile([128, 2, BW], b16, tag="xcv")
        nc.sync.dma_start(out=t[:], in_=x_conv[:, b, :, :])
        return t

    # All DMAs drain through one FIFO pipe in issue order, so issue order =
    # priority: packed consts + band-0 x_conv first (unblocks the conv
    # chain), then x row chunks just-in-time interleaved with later x_conv.
    pks1 = const.tile([128, PK1W], b16, tag="pks1")
    nc.sync.dma_start(out=pks1[:], in_=pkc1)
    w1s = pks1[:, 0:128].rearrange("p (a r) -> p a r", a=2)       # [128,2,64]
    sels = pks1[0:NP, 240:272]                                    # [112,32]
    b1s = pks1[0:RED, 272:274].bitcast(f32)                       # [64,1]
    b2s = pks1[0:NP, 274:288].bitcast(f32)                        # [112,7]

    # band-0 x_conv rides in the same first DMA as the conv weights
    xcv = {0: pks1[:, 288:736].rearrange("p (a n) -> p a n", a=2)}
    pks2 = const.tile([RED, PK2W], b16, tag="pks2")
    nc.sync.dma_start(out=pks2[:], in_=pkc2)

    def w2v(ki):
        # w2 stationary for tap ki: ki=2 rides in the first (small) const
        # DMA so the band-0 conv chain starts as early as possible
        i = KI2ORD.index(ki)
        if i == 0:
            return pks1[0:RED, 128:240]
        return pks2[0:RED, 112 * (i - 1):112 * i]
    load_chunk(0)  # rows 3:6   (band 0 clipped edge taps)
    xcv[1] = load_xcv(1)
    load_chunk(1)  # rows 6:10  (band 0 fused taps, band 1 first group)
    xcv[2] = load_xcv(2)
    load_chunk(2)  # rows 10:14 (band 1)
    xcv[3] = load_xcv(3)
    load_chunk(3)  # rows 14:18 (band 2)
    load_chunk(4)  # rows 18:26 (bands 3,4)

    def conv_band(b, ki_order=tuple(range(K))):
        # conv1: h1 = relu(W1' @ x + b1'); conv2 per ki:
        # wgt[(g,kj), ki, hw_band] = relu(W2'[ki] @ h1 + b2'[ki])
        xcb = xcv.pop(b)
        p1 = ph1.tile([RED, BW], f32, tag="p1")
        nc.tensor.matmul(p1[:], w1s[:, 0, :], xcb[:, 0, :], start=True, stop=False)
        nc.tensor.matmul(p1[:], w1s[:, 1, :], xcb[:, 1, :], start=False, stop=True)
        h1b = h1p.tile([RED, BW], b16, tag="h1b")
        nc.scalar.activation(h1b[:], p1[:], Relu, bias=b1s[:], scale=1.0)
        wgb = wgp.tile([NP, K, BW], b16, tag="wgb")
        for ki in ki_order:
            p2 = ph2.tile([NP, BW], f32, tag="p2")
            nc.tensor.matmul(p2[:], w2v(ki), h1b[:], start=True, stop=True)
            nc.scalar.activation(
                wgb[:, ki, :], p2[:], Relu, bias=b2s[:, ki:ki + 1], scale=1.0
            )
        return wgb

    def reduce_mm(po, src, pix0, pixn, start, stop, skip):
        # one kj-selection pass: po[(j,g), s, (c, pix)] += sel.T @ src
        # src: [NP, 2, pixn] i-pair product slice for out pixels
        # [pix0, pix0+pixn); po cols for chan c are at c*BW + pix.
        for p4 in range(8):
            j, s = p4 // 2, p4 % 2
            dst = po[32 * j:32 * j + 32, s, 0:2 * BW].rearrange(
                "p (c n) -> p c n", c=2)[:, :, pix0:pix0 + pixn]
            nc.tensor.matmul(
                dst,
                sels[:],
                src(p4),
                start=start,
                stop=stop,
                skip_group_check=skip,
                tile_position=(0, 32 * j),
            )

    def prod_fused(b, wg, po, k0, kl, start, stop, skip):
        r0 = BR * b + k0
        # x operand: overlapping window AP [(part),(i),(ki: row stride),
        # (pix: 4 contiguous rows)] — rows r0+kk .. r0+kk+3 per fused kk.
        v = xrt[:, :, r0:r0 + 1, :]
        xw = BassAP(
            v.tensor, v.offset,
            [list(v.ap[0]), list(v.ap[1]), [W, kl], [1, BW]],
        )
        pr = prp.tile([NP, GC, kl, BW], b16, tag="pr")
        nc.vector.tensor_tensor(
            out=pr[:],
            in0=wg[:, k0:k0 + kl, :].unsqueeze(1).broadcast_to(
                [NP, GC, kl, BW]),
            in1=xw,
            op=mult,
        )
        for kk in range(kl):
            src = lambda p4, kk=kk: pr[:, 2 * p4:2 * p4 + 2, kk, :]
            reduce_mm(po, src, 0, BW, start and kk == 0,
                      stop and kk == kl - 1, skip)

    def clip_products(b, wg, kis):
        # edge taps: multiply only the valid x rows. For band 0, tap ki<3
        # covers out pixels [(3-ki)*56, 224) from x rows 3..3+ki; for band
        # 13, tap ki>3 covers out pixels [0, (7-ki)*56) from x rows
        # 52+ki..58. Returns deferred reduce items (their matmuls must
        # execute after the band's full-range start=True matmul: psum
        # start resets the whole bank).
        pr = pcp.tile([NP, GC, 6 * W], b16, tag="prc")
        items = []
        off = 0
        for ki in kis:
            if b == 0:
                rows, r0, pix0 = ki + 1, PAD, (PAD - ki) * W
            else:
                rows, r0, pix0 = K - ki, BR * b + ki, 0
            pixn = rows * W
            v = xrt[:, :, r0:r0 + 1, :]
            xw = BassAP(
                v.tensor, v.offset,
                [list(v.ap[0]), list(v.ap[1]), [1, pixn]],
            )
            nc.vector.tensor_tensor(
                out=pr[:, :, off:off + pixn],
                in0=wg[:, ki, pix0:pix0 + pixn].unsqueeze(1).broadcast_to(
                    [NP, GC, pixn]),
                in1=xw,
                op=mult,
            )
            items.append((off, pixn, pix0, ki))
            off += pixn
        return pr, items

    def clip_reduces(po, pr, items, stop_ki):
        for off, pixn, pix0, ki in items:
            reduce_mm(
                po, lambda p4, o=off, n=pixn: pr[:, 2 * p4:2 * p4 + 2, o:o + n],
                pix0, pixn, False, ki == stop_ki, True,
            )

    wg_cur = conv_band(0, ki_order=(2, 1, 0, 3, 4, 5, 6))

    for b in range(NB):
        # stream upcoming inputs (issue order = FIFO priority)
        if b + 4 < NB:
            xcv[b + 4] = load_xcv(b + 4)
        if 5 + b < len(XCH) - 1:
            load_chunk(5 + b)

        # conv chain one band ahead: its ACT relus precede copy(b) in the
        # ACT queue, so band b+1's products never wait on the psum copy.
        wg_next = conv_band(b + 1) if b + 1 < NB else None

        # involution: ki-fused DVE products + kj reduction via selection
        # matmul; ki accumulated in PSUM.
        # i-pair p4 = i//2: strip j = p4//2 (psum partitions 32j..32j+32),
        # slot s = p4%2. channel i = 4j + 2s + c.
        po = pho.tile([128, 2, 512], f32, tag="po")  # s-slot padded to a PSUM bank

        if b == 0:
            # clipped-tap products first on the DVE (they need only x rows
            # 3:6, the first chunk); their psum reduces are deferred until
            # after the fused group's full-range start=True.
            prc, items = clip_products(b, wg_cur, (2, 1, 0))
            prod_fused(b, wg_cur, po, 3, 3, True, False, True)
            clip_reduces(po, prc, items, None)
            prod_fused(b, wg_cur, po, 6, 1, False, True, True)
        elif b == NB - 1:
            prod_fused(b, wg_cur, po, 0, 3, True, False, True)
            prod_fused(b, wg_cur, po, 3, 1, False, False, True)
            prc, items = clip_products(b, wg_cur, (4, 5, 6))
            clip_reduces(po, prc, items, 6)
        else:
            for k0, kl in KIF:
                prod_fused(b, wg_cur, po, k0, kl, k0 == 0, k0 + kl == K, False)

        # PSUM -> SBUF bf16 in one 128-partition copy (psum partitions
        # 16..31 etc hold zeros from the padded stationary), then DMA
        # compacts the 4 quadrants to HBM.
        ob = osp.tile([128, 2, 2 * BW], b16, tag="ob")
        if b < NB - 1:
            nc.scalar.copy(out=ob[:], in_=po[:, :, 0:2 * BW])
            # single out DMA of all 128 partitions (16-row pad groups
            # included; host discards them) — one issue instead of four.
            nc.sync.dma_start(
                out=out[:, b, :, :],
                in_=ob[:].rearrange("p s (c n) -> p (s c) n", c=2),
            )
        else:
            # last band: pixels [56,224) are psum-final after tap ki=5's
            # reduce, so their copy+DMA overlap the final (ki=6) tap; only
            # the [0,56) slice remains on the critical path.
            obv = ob[:].rearrange("p s (c n) -> p s c n", c=2)
            pov = po[:, :, 0:2 * BW].rearrange("p s (c n) -> p s c n", c=2)
            for a, n in ((W, BW - W), (0, W)):
                nc.scalar.copy(out=obv[:, :, :, a:a + n],
                               in_=pov[:, :, :, a:a + n])
                nc.sync.dma_start(
                    out=out[:, b, :, a:a + n],
                    in_=obv[:, :, :, a:a + n].rearrange(
                        "p s c n -> p (s c) n"),
                )

        wg_cur = wg_next

    ctx.close()


def _prep_weights(w1, b1, g1, be1, m1, v1, w2, b2, g2, be2, m2, v2):
    s1 = (g1 / np.sqrt(v1 + EPS)).astype(np.float64)
    W1p = w1.astype(np.float64) * s1[:, None]
    b1p = be1 + (b1 - m1) * (g1 / np.sqrt(v1 + EPS))
    s2 = (g2 / np.sqrt(v2 + EPS)).astype(np.float64)
    W2p = w2.astype(np.float64) * s2[:, None]
    b2p = be2 + (b2 - m2) * (g2 / np.sqrt(v2 + EPS))

    w1t = np.ascontiguousarray(
        W1p.astype(np.float32).T.reshape(2, 128, RED).transpose(1, 0, 2)
    ).astype(bf16)
    # w2t[r, ki, 7g+kj] = W2p[g*49 + ki*7 + kj, r]
    w2t = np.ascontiguousarray(
        W2p.astype(np.float32).reshape(G, K, K, RED).transpose(3, 1, 0, 2).reshape(RED, K, NP)
    ).astype(bf16)
    b2t = np.ascontiguousarray(
        b2p.astype(np.float32).reshape(G, K, K).transpose(0, 2, 1).reshape(NP, K)
    )
    selm = np.zeros((NP, 32), np.float32)
    selm[:, :G] = np.repeat(np.eye(G, dtype=np.float32), NKJ, axis=0)

    # pack consts into two bf16 blobs (f32 payloads bitcast): pkc1 has
    # everything the band-0 conv chain needs, pkc2 the remaining w2 taps
    pk1 = np.zeros((128, 288), bf16)
    pk1[:, 0:128] = w1t.reshape(128, 128)
    pk1[0:RED, 128:240] = w2t[:, KI2ORD[0], :]
    pk1[0:NP, 240:272] = selm.astype(bf16)
    pk1[0:RED, 272:274] = b1p.astype(np.float32).reshape(RED, 1).view(bf16)
    pk1[0:NP, 274:288] = b2t.view(bf16)
    pk2 = np.zeros((RED, PK2W), bf16)
    for i, ki in enumerate(KI2ORD[1:]):
        pk2[:, 112 * i:112 * (i + 1)] = w2t[:, ki, :]
    return pk1, pk2


def _prep_core(xc):
    """xc: [C, H, W] fp32 -> (x_rep bf16 [112,16,62,56] kj-shifted copies,
    x_conv bf16 [128,NB,2,BW])"""
    xw = np.zeros((G, GC, HP, WP), np.float32)
    xw[:, :, PAD:PAD + H, PAD:PAD + W] = xc.reshape(G, GC, H, W)
    arr = np.empty((G, NKJ, GC, HP, W), np.float32)
    for kj in range(NKJ):
        arr[:, kj] = xw[:, :, :, kj:kj + W]
    x_rep = arr.reshape(NP, GC, HP, W).astype(bf16)
    x_conv = np.ascontiguousarray(
        xc.reshape(2, 128, NB, BW).transpose(1, 2, 0, 3)
    ).astype(bf16)
    return x_rep, x_conv


def kernel(x, w1, b1, g1, be1, m1, v1, w2, b2, g2, be2, m2, v2, _profile=False):
    from concourse.bass_utils import run_bass_kernel_spmd

    if "nc" not in _CACHE:
        _CACHE["nc"] = _build_nc()
    nc = _CACHE["nc"]

    x = np.asarray(x, np.float32)
    pk1, pk2 = _prep_weights(
        np.asarray(w1, np.float32), np.asarray(b1, np.float32),
        np.asarray(g1, np.float32), np.asarray(be1, np.float32),
        np.asarray(m1, np.float32), np.asarray(v1, np.float32),
        np.asarray(w2, np.float32), np.asarray(b2, np.float32),
        np.asarray(g2, np.float32), np.asarray(be2, np.float32),
        np.asarray(m2, np.float32), np.asarray(v2, np.float32),
    )

    in_maps = []
    for c in range(B):
        x_rep, x_conv = _prep_core(x[c].reshape(C, H, W))
        pk1c = np.concatenate(
            [pk1, np.ascontiguousarray(x_conv[:, 0]).reshape(128, 448)],
            axis=1)
        in_maps.append({"x_rep": x_rep, "x_conv": x_conv,
                        "pkc1": pk1c, "pkc2": pk2})

    res = run_bass_kernel_spmd(
        nc, in_maps, core_ids=list(range(8)), trace=_profile
    )
    outs = []
    for c in range(B):
        arr = res.results[c]["out"].astype(np.float32)  # [128, NB, 4, BW], row p = 32j+g
        arr = arr.reshape(4, 32, NB, 4, BW)[:, :G]     # [j, g, band, i4, n]
        arr = arr.transpose(1, 0, 3, 2, 4).reshape(C, HW)  # channel = 16g+4j+i4
        outs.append(arr)
    outp = np.stack(outs, axis=0)
    if _profile:
        _CACHE["last_result"] = res
    return outp.reshape(B, C, H, W).astype(np.float32)


# revision 42
# speedup vs baseline: 1.0100x; 1.0049x over previous
"""Trainium2 Bass kernel for the Involution module (B=8, C=256, H=W=56, K=7).

Strategy (8 NeuronCores, data-parallel over batch; one batch element per core):
  - conv1x1+BN+ReLU twice on the PE (bf16, BN folded into weights in numpy).
  - Involution: partitions = (group g:16, kj:7) = 112 lanes.
    x is pre-replicated 7x (kj-shifted copies) host-side and held fully
    resident in SBUF ([112, 16, 62, 56]); row-chunk DMAs are ordered so band
    0 starts as early as possible; zero pad rows are never loaded or
    multiplied (taps are row-clipped at the top/bottom image edge).
    Per band (224 pixels = 4 output rows): products pr[(g,kj), i, ki, hw] =
    wgt * x are computed on the DVE (bf16 2x mode, the bottleneck engine at
    ~100% steady-state duty), fused over ki groups (3,3,1) to amortize
    per-instruction overhead. PE reduces over kj with a 0/1 selection matmul
    (4-way col-tiled, 32-wide zero-padded stationary so all 128 psum
    partitions are written), accumulating the 7 ki taps in PSUM.
    The conv chain runs one band ahead of the products (so the ACT-queue
    order is relu(b+1)... copy(b) and the DVE never waits on the psum-copy
    chain). One ACT copy moves PSUM->SBUF bf16 per band and one DMA writes
    it out band-major (the last band splits copy+DMA per s-slot to shorten
    the tail); host decompacts and casts.
    Consts ride in two packed blobs: pkc1 carries everything the band-0
    conv chain needs (w1, w2 tap ki=2, selection matrix, biases) so the
    first products start ~14us in; pkc2 carries the remaining w2 taps.
"""

import numpy as np
import ml_dtypes

B, C, H, W = 8, 256, 56, 56
K = 7
GC = 16
G = 16
RED = 64
K2 = 49
EPS = 1e-5
HW = H * W            # 3136
PAD = 3
HP = H + 2 * PAD      # 62
WP = W + 2 * PAD      # 62
NB = 14               # bands
BW = HW // NB         # 224 columns per band (4 output rows)
BR = 4                # rows per band
NKJ = 7
NP = G * NKJ          # 112 partitions
KIF = ((0, 3), (3, 3), (6, 1))   # fused ki groups (start, len)
# SBUF/HBM row chunks of the resident x tile (padded-row indices); chunk c
# loads rows [XCH[c], XCH[c+1]). Pad rows (0:3, 59:62) are never touched.
XCH = (3, 6, 10, 14, 18, 26, 34, 42, 50, 59)
PK1W = 736            # packed part 1 (w1, w2-ki2, sel, biases, band-0 x_conv)
PK2W = 672            # packed-consts part 2 width (w2 for other kis)
KI2ORD = (2, 0, 1, 3, 4, 5, 6)   # w2 slot order in pkc1/pkc2

bf16 = ml_dtypes.bfloat16

_CACHE = {}


def _build_nc():
    import concourse.bacc as bacc
    import concourse.tile as tile
    from concourse import mybir

    f32 = mybir.dt.float32
    b16 = mybir.dt.bfloat16

    nc = bacc.Bacc("TRN2", target_bir_lowering=False, debug=False, num_devices=8)

    x_rep = nc.dram_tensor(
        "x_rep", [NP, GC, HP, W], b16, kind="ExternalInput"
    ).ap()
    x_conv = nc.dram_tensor("x_conv", [128, NB, 2, BW], b16, kind="ExternalInput").ap()
    pkc1 = nc.dram_tensor("pkc1", [128, PK1W], b16, kind="ExternalInput").ap()
    pkc2 = nc.dram_tensor("pkc2", [RED, PK2W], b16, kind="ExternalInput").ap()
    # out rows p = 32j+g (g<16; rows 32j+16..32j+31 are zero padding);
    # free (band, i4=2s+c, hw) with channel = 16g+4j+i4
    out = nc.dram_tensor("out", [128, NB, 4, BW], b16, kind="ExternalOutput").ap()

    with tile.TileContext(nc) as tc:
        _body(tc, nc, mybir, x_rep, x_conv, pkc1, pkc2, out)

    nc.compile()
    return nc


def _body(tc, nc, mybir, x_rep, x_conv, pkc1, pkc2, out):
    from concourse.bass import AP as BassAP

    f32 = mybir.dt.float32
    b16 = mybir.dt.bfloat16
    Relu = mybir.ActivationFunctionType.Relu
    mult = mybir.AluOpType.mult

    import contextlib
    ctx = contextlib.ExitStack()
    const = ctx.enter_context(tc.tile_pool(name="const", bufs=1))
    xcp = ctx.enter_context(tc.tile_pool(name="xcp", bufs=4))
    xrp = ctx.enter_context(tc.tile_pool(name="xrp", bufs=1))
    h1p = ctx.enter_context(tc.tile_pool(name="h1p", bufs=3))
    wgp = ctx.enter_context(tc.tile_pool(name="wgp", bufs=3))
    prp = ctx.enter_context(tc.tile_pool(name="prp", bufs=3))
    pcp = ctx.enter_context(tc.tile_pool(name="pcp", bufs=1))
    osp = ctx.enter_context(tc.tile_pool(name="osp", bufs=2))
    ph1 = ctx.enter_context(tc.tile_pool(name="ph1", bufs=2, space="PSUM"))
    ph2 = ctx.enter_context(tc.tile_pool(name="ph2", bufs=2, space="PSUM"))
    pho = ctx.enter_context(tc.tile_pool(name="pho", bufs=2, space="PSUM"))

    # resident x tile: [112, 16, 62, 56]; pad rows stay unwritten (never read)
    xrt = xrp.tile([NP, GC, HP, W], b16, tag="xr")

    def load_chunk(c):
        r0, r1 = XCH[c], XCH[c + 1]
        nc.sync.dma_start(out=xrt[:, :, r0:r1, :], in_=x_rep[:, :, r0:r1, :])

    def load_xcv(b):
        t = xcp.tile([128, 2, BW], b16, tag="xcv")
        nc.sync.dma_start(out=t[:], in_=x_conv[:, b, :, :])
        return t

    # All DMAs drain through one FIFO pipe in issue order, so issue order =
    # priority: packed consts + band-0 x_conv first (unblocks the conv
    # chain), then x row chunks just-in-time interleaved with later x_conv.
    pks1 = const.tile([128, PK1W], b16, tag="pks1")
    nc.sync.dma_start(out=pks1[:], in_=pkc1)
    w1s = pks1[:, 0:128].rearrange("p (a r) -> p a r", a=2)       # [128,2,64]
    sels = pks1[0:NP, 240:272]                                    # [112,32]
    b1s = pks1[0:RED, 272:274].bitcast(f32)                       # [64,1]
    b2s = pks1[0:NP, 274:288].bitcast(f32)                        # [112,7]

    # band-0 x_conv rides in the same first DMA as the conv weights
    xcv = {0: pks1[:, 288:736].rearrange("p (a n) -> p a n", a=2)}
    pks2 = const.tile([RED, PK2W], b16, tag="pks2")
    nc.sync.dma_start(out=pks2[:], in_=pkc2)

    def w2v(ki):
        # w2 stationary for tap ki: ki=2 rides in the first (small) const
        # DMA so the band-0 conv chain starts as early as possible
        i = KI2ORD.index(ki)
        if i == 0:
            return pks1[0:RED, 128:240]
        return pks2[0:RED, 112 * (i - 1):112 * i]
    load_chunk(0)  # rows 3:6   (band 0 clipped edge taps)
    xcv[1] = load_xcv(1)
    load_chunk(1)  # rows 6:10  (band 0 fused taps, band 1 first group)
    xcv[2] = load_xcv(2)
    load_chunk(2)  # rows 10:14 (band 1)
    xcv[3] = load_xcv(3)
    load_chunk(3)  # rows 14:18 (band 2)
    load_chunk(4)  # rows 18:26 (bands 3,4)

    def conv_band(b, ki_order=tuple(range(K))):
        # conv1: h1 = relu(W1' @ x + b1'); conv2 per ki:
        # wgt[(g,kj), ki, hw_band] = relu(W2'[ki] @ h1 + b2'[ki])
        xcb = xcv.pop(b)
        p1 = ph1.tile([RED, BW], f32, tag="p1")
        nc.tensor.matmul(p1[:], w1s[:, 0, :], xcb[:, 0, :], start=True, stop=False)
        nc.tensor.matmul(p1[:], w1s[:, 1, :], xcb[:, 1, :], start=False, stop=True)
        h1b = h1p.tile([RED, BW], b16, tag="h1b")
        nc.scalar.activation(h1b[:], p1[:], Relu, bias=b1s[:], scale=1.0)
        wgb = wgp.tile([NP, K, BW], b16, tag="wgb")
        for ki in ki_order:
            p2 = ph2.tile([NP, BW], f32, tag="p2")
            nc.tensor.matmul(p2[:], w2v(ki), h1b[:], start=True, stop=True)
            nc.scalar.activation(
                wgb[:, ki, :], p2[:], Relu, bias=b2s[:, ki:ki + 1], scale=1.0
            )
        return wgb

    def reduce_mm(po, src, pix0, pixn, start, stop, skip):
        # one kj-selection pass: po[(j,g), s, (c, pix)] += sel.T @ src
        # src: [NP, 2, pixn] i-pair product slice for out pixels
        # [pix0, pix0+pixn); po cols for chan c are at c*BW + pix.
        for p4 in range(8):
            j, s = p4 // 2, p4 % 2
            dst = po[32 * j:32 * j + 32, s, 0:2 * BW].rearrange(
                "p (c n) -> p c n", c=2)[:, :, pix0:pix0 + pixn]
            nc.tensor.matmul(
                dst,
                sels[:],
                src(p4),
                start=start,
                stop=stop,
                skip_group_check=skip,
                tile_position=(0, 32 * j),
            )

    def prod_fused(b, wg, po, k0, kl, start, stop, skip):
        r0 = BR * b + k0
        # x operand: overlapping window AP [(part),(i),(ki: row stride),
        # (pix: 4 contiguous rows)] — rows r0+kk .. r0+kk+3 per fused kk.
        v = xrt[:, :, r0:r0 + 1, :]
        xw = BassAP(
            v.tensor, v.offset,
            [list(v.ap[0]), list(v.ap[1]), [W, kl], [1, BW]],
        )
        pr = prp.tile([NP, GC, kl, BW], b16, tag="pr")
        nc.vector.tensor_tensor(
            out=pr[:],
            in0=wg[:, k0:k0 + kl, :].unsqueeze(1).broadcast_to(
                [NP, GC, kl, BW]),
            in1=xw,
            op=mult,
        )
        for kk in range(kl):
            src = lambda p4, kk=kk: pr[:, 2 * p4:2 * p4 + 2, kk, :]
            reduce_mm(po, src, 0, BW, start and kk == 0,
                      stop and kk == kl - 1, skip)

    def clip_products(b, wg, kis):
        # edge taps: multiply only the valid x rows. For band 0, tap ki<3
        # covers out pixels [(3-ki)*56, 224) from x rows 3..3+ki; for band
        # 13, tap ki>3 covers out pixels [0, (7-ki)*56) from x rows
        # 52+ki..58. Returns deferred reduce items (their matmuls must
        # execute after the band's full-range start=True matmul: psum
        # start resets the whole bank).
        pr = pcp.tile([NP, GC, 6 * W], b16, tag="prc")
        items = []
        off = 0
        for ki in kis:
            if b == 0:
                rows, r0, pix0 = ki + 1, PAD, (PAD - ki) * W
            else:
                rows, r0, pix0 = K - ki, BR * b + ki, 0
            pixn = rows * W
            v = xrt[:, :, r0:r0 + 1, :]
            xw = BassAP(
                v.tensor, v.offset,
                [list(v.ap[0]), list(v.ap[1]), [1, pixn]],
            )
            nc.vector.tensor_tensor(
                out=pr[:, :, off:off + pixn],
                in0=wg[:, ki, pix0:pix0 + pixn].unsqueeze(1).broadcast_to(
                    [NP, GC, pixn]),
                in1=xw,
                op=mult,
            )
            items.append((off, pixn, pix0, ki))
            off += pixn
        return pr, items

    def clip_reduces(po, pr, items, stop_ki):
        for off, pixn, pix0, ki in items:
            reduce_mm(
                po, lambda p4, o=off, n=pixn: pr[:, 2 * p4:2 * p4 + 2, o:o + n],
                pix0, pixn, False, ki == stop_ki, True,
            )

    wg_cur = conv_band(0, ki_order=(2, 1, 0, 3, 4, 5, 6))

    for b in range(NB):
        # stream upcoming inputs (issue order = FIFO priority)
        if b + 4 < NB:
            xcv[b + 4] = load_xcv(b + 4)
        if 5 + b < len(XCH) - 1:
            load_chunk(5 + b)

        # conv chain one band ahead: its ACT relus precede copy(b) in the
        # ACT queue, so band b+1's products never wait on the psum copy.
        wg_next = conv_band(b + 1) if b + 1 < NB else None

        # involution: ki-fused DVE products + kj reduction via selection
        # matmul; ki accumulated in PSUM.
        # i-pair p4 = i//2: strip j = p4//2 (psum partitions 32j..32j+32),
        # slot s = p4%2. channel i = 4j + 2s + c.
        po = pho.tile([128, 2, 512], f32, tag="po")  # s-slot padded to a PSUM bank

        if b == 0:
            # clipped-tap products first on the DVE (they need only x rows
            # 3:6, the first chunk); their psum reduces are deferred until
            # after the fused group's full-range start=True.
            prc, items = clip_products(b, wg_cur, (2, 1, 0))
            prod_fused(b, wg_cur, po, 3, 3, True, False, True)
            clip_reduces(po, prc, items, None)
            prod_fused(b, wg_cur, po, 6, 1, False, True, True)
        elif b == NB - 1:
            prod_fused(b, wg_cur, po, 0, 3, True, False, True)
            prod_fused(b, wg_cur, po, 3, 1, False, False, True)
            prc, items = clip_products(b, wg_cur, (4, 5, 6))
            clip_reduces(po, prc, items, 6)
        else:
            for k0, kl in KIF:
                prod_fused(b, wg_cur, po, k0, kl, k0 == 0, k0 + kl == K, False)

        # PSUM -> SBUF bf16 in one 128-partition copy (psum partitions
        # 16..31 etc hold zeros from the padded stationary), then DMA
        # compacts the 4 quadrants to HBM.
        ob = osp.tile([128, 2, 2 * BW], b16, tag="ob")
        if b < NB - 1:
            nc.scalar.copy(out=ob[:], in_=po[:, :, 0:2 * BW])
            # single out DMA of all 128 partitions (16-row pad groups
            # included; host discards them) — one issue instead of four.
            nc.sync.dma_start(
                out=out[:, b, :, :],
                in_=ob[:].rearrange("p s (c n) -> p (s c) n", c=2),
            )
        else:
            # last band: split copy+DMA per s-slot so the first half's
            # transfer overlaps the second half's copy (shorter tail)
            for s in range(2):
                nc.scalar.copy(out=ob[:, s, :], in_=po[:, s, 0:2 * BW])
                nc.sync.dma_start(
                    out=out[:, b, 2 * s:2 * s + 2, :],
                    in_=ob[:, s, :].rearrange("p (c n) -> p c n", c=2),
                )

        wg_cur = wg_next

    ctx.close()


def _prep_weights(w1, b1, g1, be1, m1, v1, w2, b2, g2, be2, m2, v2):
    s1 = (g1 / np.sqrt(v1 + EPS)).astype(np.float64)
    W1p = w1.astype(np.float64) * s1[:, None]
    b1p = be1 + (b1 - m1) * (g1 / np.sqrt(v1 + EPS))
    s2 = (g2 / np.sqrt(v2 + EPS)).astype(np.float64)
    W2p = w2.astype(np.float64) * s2[:, None]
    b2p = be2 + (b2 - m2) * (g2 / np.sqrt(v2 + EPS))

    w1t = np.ascontiguousarray(
        W1p.astype(np.float32).T.reshape(2, 128, RED).transpose(1, 0, 2)
    ).astype(bf16)
    # w2t[r, ki, 7g+kj] = W2p[g*49 + ki*7 + kj, r]
    w2t = np.ascontiguousarray(
        W2p.astype(np.float32).reshape(G, K, K, RED).transpose(3, 1, 0, 2).reshape(RED, K, NP)
    ).astype(bf16)
    b2t = np.ascontiguousarray(
        b2p.astype(np.float32).reshape(G, K, K).transpose(0, 2, 1).reshape(NP, K)
    )
    selm = np.zeros((NP, 32), np.float32)
    selm[:, :G] = np.repeat(np.eye(G, dtype=np.float32), NKJ, axis=0)

    # pack consts into two bf16 blobs (f32 payloads bitcast): pkc1 has
    # everything the band-0 conv chain needs, pkc2 the remaining w2 taps
    pk1 = np.zeros((128, 288), bf16)
    pk1[:, 0:128] = w1t.reshape(128, 128)
    pk1[0:RED, 128:240] = w2t[:, KI2ORD[0], :]
    pk1[0:NP, 240:272] = selm.astype(bf16)
    pk1[0:RED, 272:274] = b1p.astype(np.float32).reshape(RED, 1).view(bf16)
    pk1[0:NP, 274:288] = b2t.view(bf16)
    pk2 = np.zeros((RED, PK2W), bf16)
    for i, ki in enumerate(KI2ORD[1:]):
        pk2[:, 112 * i:112 * (i + 1)] = w2t[:, ki, :]
    return pk1, pk2


def _prep_core(xc):
    """xc: [C, H, W] fp32 -> (x_rep bf16 [112,16,62,56] kj-shifted copies,
    x_conv bf16 [128,NB,2,BW])"""
    xw = np.zeros((G, GC, HP, WP), np.float32)
    xw[:, :, PAD:PAD + H, PAD:PAD + W] = xc.reshape(G, GC, H, W)
    arr = np.empty((G, NKJ, GC, HP, W), np.float32)
    for kj in range(NKJ):
        arr[:, kj] = xw[:, :, :, kj:kj + W]
    x_rep = arr.reshape(NP, GC, HP, W).astype(bf16)
    x_conv = np.ascontiguousarray(
        xc.reshape(2, 128, NB, BW).transpose(1, 2, 0, 3)
    ).astype(bf16)
    return x_rep, x_conv


def kernel(x, w1, b1, g1, be1, m1, v1, w2, b2, g2, be2, m2, v2, _profile=False):
    from concourse.bass_utils import run_bass_kernel_spmd

    if "nc" not in _CACHE:
        _CACHE["nc"] = _build_nc()
    nc = _CACHE["nc"]

    x = np.asarray(x, np.float32)
    pk1, pk2 = _prep_weights(
        np.asarray(w1, np.float32), np.asarray(b1, np.float32),
        np.asarray(g1, np.float32), np.asarray(be1, np.float32),
        np.asarray(m1, np.float32), np.asarray(v1, np.float32),
        np.asarray(w2, np.float32), np.asarray(b2, np.float32),
        np.asarray(g2, np.float32), np.asarray(be2, np.float32),
        np.asarray(m2, np.float32), np.asarray(v2, np.float32),
    )

    in_maps = []
    for c in range(B):
        x_rep, x_conv = _prep_core(x[c].reshape(C, H, W))
        pk1c = np.concatenate(
            [pk1, np.ascontiguousarray(x_conv[:, 0]).reshape(128, 448)],
            axis=1)
        in_maps.append({"x_rep": x_rep, "x_conv": x_conv,
                        "pkc1": pk1c, "pkc2": pk2})

    res = run_bass_kernel_spmd(
        nc, in_maps, core_ids=list(range(8)), trace=_profile
    )
    outs = []
    for c in range(B):
        arr = res.results[c]["out"].astype(np.float32)  # [128, NB, 4, BW], row p = 32j+g
        arr = arr.reshape(4, 32, NB, 4, BW)[:, :G]     # [j, g, band, i4, n]
        arr = arr.transpose(1, 0, 3, 2, 4).reshape(C, HW)  # channel = 16g+4j+i4
        outs.append(arr)
    outp = np.stack(outs, axis=0)
    if _profile:
        _CACHE["last_result"] = res
    return outp.reshape(B, C, H, W).astype(np.float32)
